# revision 22
# baseline (speedup 1.0000x reference)
"""MultiHeadSelection Trainium2 kernel.

scores[b,i,j,p] = sum_k tanh(x[b,i]@u_a[:,k] + x[b,j]@w_a[:,k] + b_s[k]) * v[k,p]

Shapes (hardcoded): x [8,256,768], u_a/w_a [768,256], b_s [256], v [256,50]
-> out [8,256,256,50] float32.

Sharding: data-parallel over batch, one batch element per NeuronCore (8 cores).

Per-core dataflow (j-major so the output DMA is linear):
  stage 1: Lb[k,i] = (x_b @ u_a)^T + b_s   (bf16, k on partitions)
           R2[k,j,2] = (x_b @ w_a)^T replicated 2x along an inner dim
           (the x2 replication gives every tensor_tensor operand a packed
           2-byte inner dim -> DVE 2x mode)
  stage 2, for each block of JB=16 j's:
    pre[k,kc,j,i] = R2[k,j] + Lb[k,i]      one DVE tensor_tensor per kc,
                                           4096-elem ops in 2x mode
    th = tanh(pre)                         one 8192-elem ACT op per block
    for ic, oct:  psum[i(128), jl(8), p] += th[k, j, ic]^T @ v[kc]
                                           (i on psum partitions)
    gpsimd copies psum -> ost[i, j, p];    sync DMAs ost -> scores with
                                           3.2KB/partition contiguous runs
"""

import numpy as np
from contextlib import ExitStack

import concourse.bass as bass
import concourse.mybir as mybir
import concourse.tile as tile
from concourse import bacc

B, S, H, K, P = 8, 256, 768, 256, 50
NCORES = 8
JB = 16            # j's per stage-2 block
OCT = 8            # j's per psum bank ([128, 8*50] f32 = 1600B <= 2KB)
KC = K // 128      # 2 k-chunks
HC = H // 128      # 6 h-chunks
NB = S // JB       # 16 blocks

F32 = mybir.dt.float32
BF16 = mybir.dt.bfloat16


def _build_nc(reps=1, ablate=()):
    ablate = set(ablate)
    # reps>1 repeats the stage-2 loop on-device (same inputs/outputs) —
    # used only for timing: wall(R) - wall(1) isolates device time from
    # the per-call host/axon dispatch overhead.
    nc = bacc.Bacc("TRN2", target_bir_lowering=False, debug=False,
                   enable_partition_id=False)

    xb = nc.dram_tensor("xb", [S, H], F32, kind="ExternalInput").ap()
    ua = nc.dram_tensor("ua", [H, K], F32, kind="ExternalInput").ap()
    wa = nc.dram_tensor("wa", [H, K], F32, kind="ExternalInput").ap()
    bs = nc.dram_tensor("bs", [K], F32, kind="ExternalInput").ap()
    vv = nc.dram_tensor("vv", [K, P], F32, kind="ExternalInput").ap()
    sc = nc.dram_tensor("scores", [S, S, P], F32, kind="ExternalOutput").ap()

    with ExitStack() as ctx:
        tc = ctx.enter_context(tile.TileContext(nc))
        singles = ctx.enter_context(tc.tile_pool(name="singles", bufs=1))

        # ---- constants ----
        v_bf = singles.tile([128, KC, P], BF16)
        bs_dma = singles.tile([128, KC], F32)
        for kc in range(KC):
            nc.sync.dma_start(out=bs_dma[:, kc:kc + 1], in_=bs[kc * 128:(kc + 1) * 128])
        # Bounce through a DVE copy so the DMA-completion wait lands on the
        # copy, not on the single-wait-slot consumer.
        bs_col = singles.tile([128, KC], F32)
        nc.vector.tensor_copy(out=bs_col, in_=bs_dma)

        Lb = singles.tile([128, KC, S], BF16)      # left^T + b_s
        R2 = singles.tile([128, KC, S, 2], BF16)   # right^T, x2 replicated

        # ---- stage 1 ----
        from concourse.masks import make_identity
        with tc.tile_pool(name="s1", bufs=1) as s1, \
             tc.tile_pool(name="psT", bufs=2, space="PSUM") as psT, \
             tc.tile_pool(name="ps1", bufs=2, space="PSUM") as ps1:
            # x loads as fp32 on the idle sync/scalar HWDGE queues (no cast
            # -> no gpsimd dependency) and is transposed on the PE; the
            # psum eviction does the fp32 -> bf16 cast. No DRAM scratch
            # round-trip, no serial XBAR transpose DMAs. u/w/v casts ride
            # gpsimd's SWDGE as single batched strided DMAs.
            x_sb = s1.tile([128, 2, H], F32)
            nc.sync.dma_start(out=x_sb[:, 0, :H // 2], in_=xb[0:128, :H // 2])
            nc.scalar.dma_start(out=x_sb[:, 1, :H // 2], in_=xb[128:256, :H // 2])
            nc.sync.dma_start(out=x_sb[:, 0, H // 2:], in_=xb[0:128, H // 2:])
            nc.scalar.dma_start(out=x_sb[:, 1, H // 2:], in_=xb[128:256, H // 2:])
            u_bf = s1.tile([128, HC, K], BF16)
            w_bf = s1.tile([128, HC, K], BF16)
            nc.gpsimd.dma_start(out=u_bf, in_=ua.rearrange("(hc p) k -> p hc k", p=128))
            nc.gpsimd.dma_start(out=w_bf, in_=wa.rearrange("(hc p) k -> p hc k", p=128))
            nc.gpsimd.dma_start(out=v_bf, in_=vv.rearrange("(kc p) q -> p kc q", p=128))

            ident = s1.tile([128, 128], F32)
            make_identity(nc, ident)
            x_T = s1.tile([128, HC, S], BF16)
            for hc in range(HC):
                for ic in range(2):
                    pst = psT.tile([128, 128], F32, tag="pst")
                    nc.tensor.transpose(pst, in_=x_sb[:, ic, hc * 128:(hc + 1) * 128],
                                        identity=ident)
                    if ic == 0:
                        nc.scalar.activation(out=x_T[:, hc, 0:128], in_=pst,
                                             func=mybir.ActivationFunctionType.Copy)
                    else:
                        nc.vector.tensor_copy(out=x_T[:, hc, 128:256], in_=pst)

            for kc in range(KC):
                ps_r = ps1.tile([128, S], F32, tag="ps_r")
                ps_l = ps1.tile([128, S], F32, tag="ps_l")
                for hc in range(HC):
                    nc.tensor.matmul(ps_r, lhsT=w_bf[:, hc, kc * 128:(kc + 1) * 128],
                                     rhs=x_T[:, hc, :], start=(hc == 0), stop=(hc == HC - 1))
                for hc in range(HC):
                    nc.tensor.matmul(ps_l, lhsT=u_bf[:, hc, kc * 128:(kc + 1) * 128],
                                     rhs=x_T[:, hc, :], start=(hc == 0), stop=(hc == HC - 1))
                # Lb = ps_l + b_s (ACT Identity with per-partition bias)
                nc.scalar.activation(out=Lb[:, kc, :], in_=ps_l,
                                     func=mybir.ActivationFunctionType.Identity,
                                     bias=bs_col[:, kc:kc + 1])
                # R2[k, j, il] = ps_r[k, j] for il in {0, 1}
                r_in = ps_r.unsqueeze(2).broadcast_to((128, S, 2))
                nc.scalar.activation(out=R2[:, kc], in_=r_in,
                                     func=mybir.ActivationFunctionType.Copy)

        # ---- stage 2 ----
        work = ctx.enter_context(tc.tile_pool(name="work", bufs=2))
        ostp = ctx.enter_context(tc.tile_pool(name="ostp", bufs=2))
        pso = ctx.enter_context(tc.tile_pool(name="pso", bufs=8, space="PSUM"))

        # 8-wide half-segments at the ends fill the ACT pipe sooner and let
        # the drain (matmul/evict/DMA of the final tanh) be half as long.
        segs = [(0, 8), (8, 8)] + [(16 * b, 16) for b in range(1, NB - 1)] \
            + [(S - 16, 8), (S - 8, 8)]
        blocks = [sg for _ in range(reps) for sg in segs]
        for bi, (j0, sl) in enumerate(blocks):
            pre = work.tile([128, KC, sl, S], BF16, tag=f"pre{sl}")
            th = work.tile([128, KC, sl, S], BF16, tag=f"th{sl}")
            for kc in range(KC):
                # pre[k, j, ih, il] = R2[k, j, il] + Lb[k, ih*2 + il]
                # (ih, il) = i split 128x2 so every operand has a packed
                # 2-byte inner dim -> DVE 2x mode.
                in0 = R2[:, kc, j0:j0 + sl, :].unsqueeze(2).broadcast_to(
                    (128, sl, 128, 2))
                in1 = Lb[:, kc, :].rearrange("p (ih il) -> p ih il", il=2) \
                    .unsqueeze(1).broadcast_to((128, sl, 128, 2))
                outv = pre[:, kc].rearrange("p j (ih il) -> p j ih il", il=2)
                nc.vector.tensor_add(out=outv, in0=in0, in1=in1)
            if "no_act" not in ablate:
                if bi == 0:
                    # kc-split so the very first ACT op starts earliest.
                    for kc in range(KC):
                        nc.scalar.activation(out=th[:, kc], in_=pre[:, kc],
                                             func=mybir.ActivationFunctionType.Tanh)
                else:
                    nc.scalar.activation(out=th, in_=pre,
                                         func=mybir.ActivationFunctionType.Tanh)
            src = pre if "no_act" in ablate else th
            ost = ostp.tile([128, 2, sl, P], F32, tag=f"ost{sl}")
            for ic in range(2):
                for oc in range(sl // OCT):
                    po = pso.tile([128, OCT, P], F32, tag="po")
                    if "no_mm" not in ablate:
                        for jl in range(OCT):
                            jloc = oc * OCT + jl
                            for kc in range(KC):
                                nc.tensor.matmul(
                                    po[:, jl, :],
                                    lhsT=src[:, kc, jloc, ic * 128:(ic + 1) * 128],
                                    rhs=v_bf[:, kc, :],
                                    start=(kc == 0), stop=(kc == KC - 1))
                    # (GPSIMD cannot read PSUM on TRN2, so DVE evicts.)
                    nc.vector.tensor_copy(
                        out=ost[:, ic, oc * OCT:(oc + 1) * OCT, :], in_=po)
                if "no_dma" not in ablate:
                    nc.sync.dma_start(
                        out=sc[ic * 128:(ic + 1) * 128, j0:j0 + sl, :],
                        in_=ost[:, ic])

    return nc


_RUNNERS = {}


def _get_runner(reps=1, ablate=()):
    key = (reps, tuple(sorted(ablate)))
    if key in _RUNNERS:
        return _RUNNERS[key]
    import jax
    from jax.sharding import Mesh, PartitionSpec
    from jax.experimental.shard_map import shard_map
    from concourse.bass2jax import install_neuronx_cc_hook, _bass_exec_p

    install_neuronx_cc_hook()
    nc = _build_nc(reps=reps, ablate=ablate)
    if not nc.is_finalized():
        nc.finalize()

    in_names, out_names, out_avals = [], [], []
    for alloc in nc.m.functions[0].allocations:
        if not isinstance(alloc, mybir.MemoryLocationSet):
            continue
        if alloc.kind not in ("ExternalInput", "ExternalOutput"):
            continue
        name = alloc.memorylocations[0].name
        if alloc.kind == "ExternalInput":
            in_names.append(name)
        else:
            out_names.append(name)
            out_avals.append(jax.core.ShapedArray(tuple(alloc.tensor_shape),
                                                  mybir.dt.np(alloc.dtype)))
    n_params = len(in_names)
    all_in_names = tuple(in_names + out_names)

    def _body(*args):
        outs = _bass_exec_p.bind(
            *args,
            out_avals=tuple(out_avals),
            in_names=all_in_names,
            out_names=tuple(out_names),
            lowering_input_output_aliases=(),
            sim_require_finite=True,
            sim_require_nnan=True,
            nc=nc,
        )
        return tuple(outs)

    devices = jax.devices()[:NCORES]
    assert len(devices) == NCORES, f"need {NCORES} cores, got {len(devices)}"
    mesh = Mesh(np.asarray(devices), ("core",))
    nin = n_params + len(out_names)
    fn = jax.jit(
        shard_map(_body, mesh=mesh,
                  in_specs=(PartitionSpec("core"),) * nin,
                  out_specs=(PartitionSpec("core"),) * len(out_names),
                  check_rep=False),
        keep_unused=True,
    )
    _RUNNERS[key] = (fn, in_names, out_names, out_avals, mesh)
    return _RUNNERS[key]


def _concat_args(x, u_a, w_a, b_s, v, in_names, out_avals):
    x = np.ascontiguousarray(np.asarray(x, dtype=np.float32))
    u_a = np.asarray(u_a, dtype=np.float32)
    w_a = np.asarray(w_a, dtype=np.float32)
    b_s = np.asarray(b_s, dtype=np.float32)
    v = np.asarray(v, dtype=np.float32)
    per = {
        "xb": x.reshape(NCORES * S, H),
        "ua": np.tile(u_a, (NCORES, 1)),
        "wa": np.tile(w_a, (NCORES, 1)),
        "bs": np.tile(b_s, NCORES),
        "vv": np.tile(v, (NCORES, 1)),
    }
    args = [per[n] for n in in_names]
    args += [np.zeros((NCORES * a.shape[0], *a.shape[1:]), a.dtype) for a in out_avals]
    return args


def kernel(x, u_a, w_a, b_s, v):
    fn, in_names, out_names, out_avals, mesh = _get_runner()
    args = _concat_args(x, u_a, w_a, b_s, v, in_names, out_avals)
    outs = fn(*args)
    scores = np.asarray(outs[out_names.index("scores")])
    return scores.reshape(B, S, S, P)


def _timed_calls(reps, x, u_a, w_a, b_s, v, iters, ablate=()):
    import time
    import jax
    from jax.sharding import NamedSharding, PartitionSpec

    fn, in_names, out_names, out_avals, mesh = _get_runner(reps=reps, ablate=ablate)
    args = _concat_args(x, u_a, w_a, b_s, v, in_names, out_avals)
    sh = NamedSharding(mesh, PartitionSpec("core"))
    dargs = [jax.device_put(a, sh) for a in args]
    for _ in range(3):  # warmup (also triggers compile)
        outs = fn(*dargs)
    jax.block_until_ready(outs)
    times = []
    for _ in range(iters):
        t0 = time.perf_counter()
        out = fn(*dargs)
        jax.block_until_ready(out)
        times.append(time.perf_counter() - t0)
    return times


def bench(x, u_a, w_a, b_s, v, iters=10, r_hi=5):
    """Estimate on-device time of one full computation.

    Runs NEFFs with the stage-2 loop executed once and r_hi times; the
    difference isolates device time from per-call host/axon dispatch
    overhead. Returns seconds for one computation (stage2 delta-based).
    """
    t1 = _timed_calls(1, x, u_a, w_a, b_s, v, iters)
    th = _timed_calls(r_hi, x, u_a, w_a, b_s, v, iters)
    t1m, thm = min(t1), min(th)
    stage2 = (thm - t1m) / (r_hi - 1)
    return stage2, dict(t_r1=t1m, t_rhi=thm, r_hi=r_hi,
                        med_r1=sorted(t1)[len(t1) // 2],
                        med_rhi=sorted(th)[len(th) // 2])
